# revision 14
# baseline (speedup 1.0000x reference)
"""Trainium2 Bass kernel for nn_AttentionFusion (8-core data-parallel over B).

Reference computation per batch b:
    p_proj = X @ W_p + b_p                      # (N, C)
    c_proj = CF @ W_c + b_c                     # (NC, C)
    S      = p_proj @ c_proj.T                  # (N, NC)
    W      = softmax(S, axis=-1)
    out    = X + W @ CF                         # (N, C)

Algebraic refactor used here (exact in real arithmetic):
    S = X @ M + 1·t  with  M = W_p @ c_proj.T (C×NC),  t = b_p @ c_proj.T (NC)
so the (N,C)x(C,C) projection matmul disappears; per-core PE work is the
scores matmul, the weighted sum, and the X transposes the PE needs anyway.

Pipeline layout: scores are computed TRANSPOSED (S^T [NC, rows], moving dim
= 512 rows) so exp's +t bias is per-partition and exp(S^T) feeds the
weighted-sum matmul directly as the stationary operand.  A ones-column
appended to CF yields the softmax normalizer in the same matmul.

Sharding: B=8 batches -> one batch per NeuronCore, weights replicated.
"""

import numpy as np

B, N, NC, C = 8, 16384, 64, 256
P = 128  # SBUF partitions
SUPER_ROWS = 1024  # rows per DMA supertile (row = s*1024 + p*8 + j)
JCHUNK = SUPER_ROWS // P  # 8 row-chunks per supertile
HALF = 4  # chunks per scores tile (4*128 = 512 rows)
NSUPER = N // SUPER_ROWS

_CACHE = {}


def _split_multiwait_ctrl(nc, mybir):
    """This toolchain's walrus accepts only ONE sync wait per instruction,
    but Tile's scheduler attaches one wait per depended-on proc.  Keep the
    last wait on the instruction and hoist the excess onto single-wait NoOps
    inserted immediately before it on the same engine (same sequencer order,
    identical blocking semantics)."""
    for f in nc.m.functions:
        for bb in f.blocks:
            insts = bb.instructions
            new_list = []
            changed = False
            for inst in insts:
                si = inst.sync_info
                if si is not None and si.on_wait and len(si.on_wait) > 1:
                    waits = list(si.on_wait)
                    for w in waits[:-1]:
                        nop = mybir.InstNoOp(
                            name=nc.get_next_instruction_name(),
                            engine=inst.engine,
                            sync_info=mybir.SyncInfo(on_wait=[w], on_update=[]),
                            bass_nofuse=True,
                        )
                        nc.register_instruction(nop, overwrite=True)
                        new_list.append(nop)
                        changed = True
                    inst.sync_info = mybir.SyncInfo(
                        on_wait=[waits[-1]], on_update=list(si.on_update or [])
                    )
                new_list.append(inst)
            if changed:
                bb.instructions[:] = new_list
    return nc


def _build():
    from contextlib import ExitStack

    import concourse.bass as bass
    import concourse.mybir as mybir
    import concourse.tile as tile
    from concourse.masks import make_identity

    f32 = mybir.dt.float32
    f32r = mybir.dt.float32r
    Exp = mybir.ActivationFunctionType.Exp

    nc = bass.Bass("TRN2", target_bir_lowering=False, debug=False)
    x = nc.declare_dram_parameter("x", [N, C], f32, isOutput=False)
    cf = nc.declare_dram_parameter("cf", [NC, C], f32, isOutput=False)
    wp = nc.declare_dram_parameter("wp", [C, C], f32, isOutput=False)
    bp = nc.declare_dram_parameter("bp", [C], f32, isOutput=False)
    wc = nc.declare_dram_parameter("wc", [C, C], f32, isOutput=False)
    bc = nc.declare_dram_parameter("bc", [C], f32, isOutput=False)
    out = nc.declare_dram_parameter("out", [N, C], f32, isOutput=True)

    KC = C // P  # 2 contraction chunks of 128 over the C dim

    with tile.TileContext(nc) as tc:
        with (
            tc.tile_pool(name="const", bufs=1) as const,
            tc.tile_pool(name="xin", bufs=4) as xin,
            tc.tile_pool(name="oout", bufs=4) as oout,
            tc.tile_pool(name="work", bufs=4) as work,
        ):
            setup_stack = ExitStack()
            setup_ps = setup_stack.enter_context(
                tc.tile_pool(name="setup_ps", bufs=2, space="PSUM")
            )
            # ---------------- setup: identity, weights, M, t, cf_aug -------
            ident = const.tile([P, P], f32)
            make_identity(nc, ident)

            wp_sb = const.tile([P, KC, C], f32)
            nc.sync.dma_start(wp_sb, wp.rearrange("(o p) d -> p o d", p=P))
            wc_sb = const.tile([P, KC, C], f32)
            nc.sync.dma_start(wc_sb, wc.rearrange("(o p) d -> p o d", p=P))
            bp_sb = const.tile([P, KC], f32)
            nc.sync.dma_start(bp_sb, bp.rearrange("(o p) -> p o", p=P))
            bc_sb = const.tile([P, KC], f32)
            nc.sync.dma_start(bc_sb, bc.rearrange("(o p) -> p o", p=P))
            cf_sb = const.tile([NC, C], f32)
            nc.sync.dma_start(cf_sb, cf.ap())

            # cfT[c, k] = CF[k, c]   as [128, KC, NC]
            cfT = const.tile([P, KC, NC], f32)
            for i in range(KC):
                pt = setup_ps.tile([P, NC], f32, tag="setup")
                nc.tensor.transpose(pt, cf_sb[:, bass.ts(i, P)], ident[:NC, :NC])
                nc.vector.tensor_copy(cfT[:, i, :], pt)

            # c_projT[d, k] = sum_c W_c[c,d] cfT[c,k] + b_c[d]   as [128, KC, NC]
            cprojT = const.tile([P, KC, NC], f32)
            for i in range(KC):
                pt = setup_ps.tile([P, NC], f32, tag="setup")
                for k in range(KC):
                    nc.tensor.matmul(
                        pt,
                        wc_sb[:, k, bass.ts(i, P)],
                        cfT[:, k, :],
                        start=(k == 0),
                        stop=(k == KC - 1),
                    )
                nc.vector.tensor_scalar_add(cprojT[:, i, :], pt, bc_sb[:, i : i + 1])

            # wpT[d, c] = W_p[c, d]   as [128, KC, C]
            wpT = const.tile([P, KC, C], f32)
            for i in range(KC):  # d chunk
                for j in range(KC):  # c chunk
                    pt = setup_ps.tile([P, P], f32, tag="setup")
                    nc.tensor.transpose(pt, wp_sb[:, j, bass.ts(i, P)], ident)
                    nc.vector.tensor_copy(wpT[:, i, bass.ts(j, P)], pt)

            # M[c, k] = sum_d W_p[c,d] c_projT[d,k]   as [128, KC, NC]
            mc_sb = const.tile([P, KC, NC], f32)
            for i in range(KC):  # c chunk
                pt = setup_ps.tile([P, NC], f32, tag="setup")
                for k in range(KC):  # d chunk
                    nc.tensor.matmul(
                        pt,
                        wpT[:, k, bass.ts(i, P)],
                        cprojT[:, k, :],
                        start=(k == 0),
                        stop=(k == KC - 1),
                    )
                nc.vector.tensor_copy(mc_sb[:, i, :], pt)

            # tT[k] = sum_d c_projT[d,k] b_p[d]   as [NC, 1] (exp bias)
            t_ps = setup_ps.tile([NC, 1], f32, tag="setup_t")
            for k in range(KC):
                nc.tensor.matmul(
                    t_ps,
                    cprojT[:, k, :],
                    bp_sb[:, k : k + 1],
                    start=(k == 0),
                    stop=(k == KC - 1),
                )
            tT = const.tile([NC, 1], f32)
            nc.vector.tensor_copy(tT, t_ps)

            # cf_aug[k, :] = [CF[k, :] | 1 | 1]   as [NC, C+2]
            # float32r: the weighted-sum matmul runs fp32r (1 cyc/row at
            # free>=256); the ISA needs even element counts, hence C+2.
            cf_aug = const.tile([NC, C + 2], f32r)
            nc.vector.tensor_copy(cf_aug[:, :C], cf_sb)
            one = const.tile([NC, 1], f32)
            nc.vector.memset(one, 1.0)
            nc.vector.tensor_copy(cf_aug[:, C : C + 1], one)
            nc.vector.tensor_copy(cf_aug[:, C + 1 : C + 2], one)

            # ---------------- main loop --------------------------------------
            setup_stack.close()
            ps_stack = ExitStack()
            ps_xt = ps_stack.enter_context(
                tc.tile_pool(name="ps_xt", bufs=1, space="PSUM")
            )
            ps_sc = ps_stack.enter_context(
                tc.tile_pool(name="ps_sc", bufs=3, space="PSUM")
            )
            ps_ws = ps_stack.enter_context(
                tc.tile_pool(name="ps_ws", bufs=3, space="PSUM")
            )
            x_view = x.rearrange("(s p j) c -> s p j c", p=P, j=JCHUNK)
            o_view = out.rearrange("(s p j) c -> s p j c", p=P, j=JCHUNK)

            RW = HALF * P  # 512 rows per scores tile

            for s in range(NSUPER):
                x_tile = xin.tile([P, JCHUNK, C], f32)
                nc.sync.dma_start(x_tile, x_view[s])
                o_tile = oout.tile([P, JCHUNK, C], f32)

                for h in range(JCHUNK // HALF):
                    # X^T for 512 rows: per c-chunk k, [128, 512] (free =
                    # jj*128 + p  <->  row s*1024 + p*8 + (h*HALF+jj))
                    xt_ps = [
                        ps_xt.tile([P, RW], f32, tag=f"xt{k}", name=f"xt_ps{k}")
                        for k in range(KC)
                    ]
                    for jj in range(HALF):
                        j = h * HALF + jj
                        for k in range(KC):
                            nc.tensor.transpose(
                                xt_ps[k][:, bass.ts(jj, P)],
                                x_tile[:, j, bass.ts(k, P)],
                                ident,
                            )
                    xt_sb = [
                        work.tile([P, RW], f32, tag=f"xt_sb{k}", name=f"xt_sb{k}")
                        for k in range(KC)
                    ]
                    # alternate the PSUM->SBUF copies between DVE and ACT
                    nc.vector.tensor_copy(xt_sb[0], xt_ps[0])
                    nc.scalar.copy(xt_sb[1], xt_ps[1])

                    # S^T[k, r] = sum_c M[c,k] X[r,c]
                    sc_ps = ps_sc.tile([NC, RW], f32, tag="sc")
                    for k in range(KC):
                        nc.tensor.matmul(
                            sc_ps,
                            mc_sb[:, k, :],
                            xt_sb[k],
                            start=(k == 0),
                            stop=(k == KC - 1),
                        )

                    # expT = exp(S^T + t)  (f32r: feeds the fp32r matmul)
                    expT = work.tile([NC, RW], f32r, tag="expT")
                    nc.scalar.activation(expT, sc_ps, Exp, bias=tT)

                    for jj in range(HALF):
                        j = h * HALF + jj
                        x_chunk = x_tile[:, j, :]

                        # weighted[r, c] = sum_k expT[k,r] [CF|1][k,c]
                        ws_ps = ps_ws.tile([P, C + 2], f32, tag="ws")
                        nc.tensor.matmul(
                            ws_ps,
                            expT[:, bass.ts(jj, P)],
                            cf_aug,
                            start=True,
                            stop=True,
                        )

                        # normalize + residual (round-robin the engines:
                        # ACT and DVE split the PSUM-normalize, GPSIMD does
                        # most of the residual adds)
                        recip = work.tile([P, 1], f32, tag="recip")
                        nc.vector.reciprocal(recip, ws_ps[:, C : C + 1])
                        tmp = work.tile([P, C], f32, tag="tmp")
                        if j % 2 == 0:
                            nc.scalar.mul(tmp, ws_ps[:, :C], recip)
                        else:
                            nc.vector.tensor_scalar_mul(tmp, ws_ps[:, :C], recip)
                        nc.gpsimd.tensor_add(o_tile[:, j, :], tmp, x_chunk)

                nc.sync.dma_start(o_view[s], o_tile)

            ps_stack.close()

    return _split_multiwait_ctrl(nc, mybir)


def _get_nc():
    if "nc" not in _CACHE:
        _CACHE["nc"] = _build()
    return _CACHE["nc"]


def run(inputs, trace=False):
    from concourse.bass_utils import run_bass_kernel_spmd

    nc = _get_nc()
    pf = np.ascontiguousarray(np.asarray(inputs["point_features"], dtype=np.float32))
    cfeat = np.ascontiguousarray(
        np.asarray(inputs["centroid_features"], dtype=np.float32)
    )
    wp = np.ascontiguousarray(np.asarray(inputs["W_p"], dtype=np.float32))
    bp = np.ascontiguousarray(np.asarray(inputs["b_p"], dtype=np.float32))
    wc = np.ascontiguousarray(np.asarray(inputs["W_c"], dtype=np.float32))
    bc = np.ascontiguousarray(np.asarray(inputs["b_c"], dtype=np.float32))

    in_maps = [
        {"x": pf[b], "cf": cfeat[b], "wp": wp, "bp": bp, "wc": wc, "bc": bc}
        for b in range(B)
    ]
    res = run_bass_kernel_spmd(nc, in_maps, core_ids=list(range(B)), trace=trace)
    out = np.stack([res.results[b]["out"] for b in range(B)], axis=0)
    return out, res


def kernel(**inputs) -> np.ndarray:
    out, _ = run(inputs, trace=False)
    return out


# revision 16
# speedup vs baseline: 1.0410x; 1.0410x over previous
"""Trainium2 Bass kernel for nn_AttentionFusion (8-core data-parallel over B).

Reference computation per batch b:
    p_proj = X @ W_p + b_p                      # (N, C)
    c_proj = CF @ W_c + b_c                     # (NC, C)
    S      = p_proj @ c_proj.T                  # (N, NC)
    W      = softmax(S, axis=-1)
    out    = X + W @ CF                         # (N, C)

Algebraic refactor used here (exact in real arithmetic):
    S = X @ M + 1·t  with  M = W_p @ c_proj.T (C×NC),  t = b_p @ c_proj.T (NC)
so the (N,C)x(C,C) projection matmul disappears; per-core PE work is the
scores matmul, the weighted sum, and the X transposes the PE needs anyway.

Pipeline layout: scores are computed TRANSPOSED (S^T [NC, rows], moving dim
= 512 rows) so exp's +t bias is per-partition and exp(S^T) feeds the
weighted-sum matmul directly as the stationary operand.  A ones-column
appended to CF yields the softmax normalizer in the same matmul.

Sharding: B=8 batches -> one batch per NeuronCore, weights replicated.
"""

import numpy as np

B, N, NC, C = 8, 16384, 64, 256
P = 128  # SBUF partitions
SUPER_ROWS = 1024  # rows per DMA supertile (row = s*1024 + p*8 + j)
JCHUNK = SUPER_ROWS // P  # 8 row-chunks per supertile
HALF = 4  # chunks per scores tile (4*128 = 512 rows)
NSUPER = N // SUPER_ROWS

_CACHE = {}


def _split_multiwait_ctrl(nc, mybir):
    """This toolchain's walrus accepts only ONE sync wait per instruction,
    but Tile's scheduler attaches one wait per depended-on proc.  Keep the
    last wait on the instruction and hoist the excess onto single-wait NoOps
    inserted immediately before it on the same engine (same sequencer order,
    identical blocking semantics)."""
    for f in nc.m.functions:
        for bb in f.blocks:
            insts = bb.instructions
            new_list = []
            changed = False
            for inst in insts:
                si = inst.sync_info
                if si is not None and si.on_wait and len(si.on_wait) > 1:
                    waits = list(si.on_wait)
                    for w in waits[:-1]:
                        nop = mybir.InstNoOp(
                            name=nc.get_next_instruction_name(),
                            engine=inst.engine,
                            sync_info=mybir.SyncInfo(on_wait=[w], on_update=[]),
                            bass_nofuse=True,
                        )
                        nc.register_instruction(nop, overwrite=True)
                        new_list.append(nop)
                        changed = True
                    inst.sync_info = mybir.SyncInfo(
                        on_wait=[waits[-1]], on_update=list(si.on_update or [])
                    )
                new_list.append(inst)
            if changed:
                bb.instructions[:] = new_list
    return nc


def _build():
    from contextlib import ExitStack

    import concourse.bass as bass
    import concourse.mybir as mybir
    import concourse.tile as tile
    from concourse.masks import make_identity

    f32 = mybir.dt.float32
    f32r = mybir.dt.float32r
    Exp = mybir.ActivationFunctionType.Exp

    nc = bass.Bass("TRN2", target_bir_lowering=False, debug=False)
    x = nc.declare_dram_parameter("x", [N, C], f32, isOutput=False)
    cf = nc.declare_dram_parameter("cf", [NC, C], f32, isOutput=False)
    wp = nc.declare_dram_parameter("wp", [C, C], f32, isOutput=False)
    bp = nc.declare_dram_parameter("bp", [C], f32, isOutput=False)
    wc = nc.declare_dram_parameter("wc", [C, C], f32, isOutput=False)
    bc = nc.declare_dram_parameter("bc", [C], f32, isOutput=False)
    out = nc.declare_dram_parameter("out", [N, C], f32, isOutput=True)

    KC = C // P  # 2 contraction chunks of 128 over the C dim

    with tile.TileContext(nc) as tc:
        with (
            tc.tile_pool(name="const", bufs=1) as const,
            tc.tile_pool(name="xin", bufs=4) as xin,
            tc.tile_pool(name="oout", bufs=4) as oout,
            tc.tile_pool(name="work", bufs=4) as work,
        ):
            setup_stack = ExitStack()
            setup_ps = setup_stack.enter_context(
                tc.tile_pool(name="setup_ps", bufs=2, space="PSUM")
            )
            # ---------------- setup: identity, weights, M, t, cf_aug -------
            ident = const.tile([P, P], f32)
            make_identity(nc, ident)

            wp_sb = const.tile([P, KC, C], f32)
            nc.gpsimd.dma_start(wp_sb, wp.rearrange("(o p) d -> p o d", p=P))
            wc_sb = const.tile([P, KC, C], f32)
            nc.gpsimd.dma_start(wc_sb, wc.rearrange("(o p) d -> p o d", p=P))
            bp_sb = const.tile([P, KC], f32)
            nc.gpsimd.dma_start(bp_sb, bp.rearrange("(o p) -> p o", p=P))
            bc_sb = const.tile([P, KC], f32)
            nc.gpsimd.dma_start(bc_sb, bc.rearrange("(o p) -> p o", p=P))
            cf_sb = const.tile([NC, C], f32)
            nc.gpsimd.dma_start(cf_sb, cf.ap())

            # cfT[c, k] = CF[k, c]   as [128, KC, NC]
            cfT = const.tile([P, KC, NC], f32)
            for i in range(KC):
                pt = setup_ps.tile([P, NC], f32, tag="setup")
                nc.tensor.transpose(pt, cf_sb[:, bass.ts(i, P)], ident[:NC, :NC])
                nc.vector.tensor_copy(cfT[:, i, :], pt)

            # c_projT[d, k] = sum_c W_c[c,d] cfT[c,k] + b_c[d]   as [128, KC, NC]
            cprojT = const.tile([P, KC, NC], f32)
            for i in range(KC):
                pt = setup_ps.tile([P, NC], f32, tag="setup")
                for k in range(KC):
                    nc.tensor.matmul(
                        pt,
                        wc_sb[:, k, bass.ts(i, P)],
                        cfT[:, k, :],
                        start=(k == 0),
                        stop=(k == KC - 1),
                    )
                nc.vector.tensor_scalar_add(cprojT[:, i, :], pt, bc_sb[:, i : i + 1])

            # wpT[d, c] = W_p[c, d]   as [128, KC, C]
            wpT = const.tile([P, KC, C], f32)
            for i in range(KC):  # d chunk
                for j in range(KC):  # c chunk
                    pt = setup_ps.tile([P, P], f32, tag="setup")
                    nc.tensor.transpose(pt, wp_sb[:, j, bass.ts(i, P)], ident)
                    nc.vector.tensor_copy(wpT[:, i, bass.ts(j, P)], pt)

            # M[c, k] = sum_d W_p[c,d] c_projT[d,k]   as [128, KC, NC]
            mc_sb = const.tile([P, KC, NC], f32)
            for i in range(KC):  # c chunk
                pt = setup_ps.tile([P, NC], f32, tag="setup")
                for k in range(KC):  # d chunk
                    nc.tensor.matmul(
                        pt,
                        wpT[:, k, bass.ts(i, P)],
                        cprojT[:, k, :],
                        start=(k == 0),
                        stop=(k == KC - 1),
                    )
                nc.vector.tensor_copy(mc_sb[:, i, :], pt)

            # tT[k] = sum_d c_projT[d,k] b_p[d]   as [NC, 1] (exp bias)
            t_ps = setup_ps.tile([NC, 1], f32, tag="setup_t")
            for k in range(KC):
                nc.tensor.matmul(
                    t_ps,
                    cprojT[:, k, :],
                    bp_sb[:, k : k + 1],
                    start=(k == 0),
                    stop=(k == KC - 1),
                )
            tT = const.tile([NC, 1], f32)
            nc.vector.tensor_copy(tT, t_ps)

            # cf_aug[k, :] = [CF[k, :] | 1 | 1]   as [NC, C+2]
            # float32r: the weighted-sum matmul runs fp32r (1 cyc/row at
            # free>=256); the ISA needs even element counts, hence C+2.
            cf_aug = const.tile([NC, C + 2], f32r)
            nc.vector.tensor_copy(cf_aug[:, :C], cf_sb)
            one = const.tile([NC, 1], f32)
            nc.vector.memset(one, 1.0)
            nc.vector.tensor_copy(cf_aug[:, C : C + 1], one)
            nc.vector.tensor_copy(cf_aug[:, C + 1 : C + 2], one)

            # ---------------- main loop --------------------------------------
            setup_stack.close()
            ps_stack = ExitStack()
            ps_xt = ps_stack.enter_context(
                tc.tile_pool(name="ps_xt", bufs=1, space="PSUM")
            )
            ps_sc = ps_stack.enter_context(
                tc.tile_pool(name="ps_sc", bufs=2, space="PSUM")
            )
            ps_ws = ps_stack.enter_context(
                tc.tile_pool(name="ps_ws", bufs=4, space="PSUM")
            )
            x_view = x.rearrange("(s p j) c -> s p j c", p=P, j=JCHUNK)
            o_view = out.rearrange("(s p j) c -> s p j c", p=P, j=JCHUNK)

            RW = HALF * P  # 512 rows per scores tile

            for s in range(NSUPER):
                x_tile = xin.tile([P, JCHUNK, C], f32)
                nc.sync.dma_start(x_tile[:, :HALF], x_view[s, :, :HALF])
                nc.sync.dma_start(x_tile[:, HALF:], x_view[s, :, HALF:])
                o_tile = oout.tile([P, JCHUNK, C], f32)

                for h in range(JCHUNK // HALF):
                    # X^T for 512 rows: per c-chunk k, [128, 512] (free =
                    # jj*128 + p  <->  row s*1024 + p*8 + (h*HALF+jj))
                    xt_ps = [
                        ps_xt.tile([P, RW], f32, tag=f"xt{k}", name=f"xt_ps{k}")
                        for k in range(KC)
                    ]
                    for jj in range(HALF):
                        j = h * HALF + jj
                        for k in range(KC):
                            nc.tensor.transpose(
                                xt_ps[k][:, bass.ts(jj, P)],
                                x_tile[:, j, bass.ts(k, P)],
                                ident,
                            )
                    xt_sb = [
                        work.tile([P, RW], f32, tag=f"xt_sb{k}", name=f"xt_sb{k}")
                        for k in range(KC)
                    ]
                    # alternate the PSUM->SBUF copies between DVE and ACT
                    nc.vector.tensor_copy(xt_sb[0], xt_ps[0])
                    nc.scalar.copy(xt_sb[1], xt_ps[1])

                    # S^T[k, r] = sum_c M[c,k] X[r,c]
                    sc_ps = ps_sc.tile([NC, RW], f32, tag="sc")
                    for k in range(KC):
                        nc.tensor.matmul(
                            sc_ps,
                            mc_sb[:, k, :],
                            xt_sb[k],
                            start=(k == 0),
                            stop=(k == KC - 1),
                        )

                    # expT = exp(S^T + t)  (f32r: feeds the fp32r matmul)
                    expT = work.tile([NC, RW], f32r, tag="expT")
                    nc.scalar.activation(expT, sc_ps, Exp, bias=tT)

                    for jj in range(HALF):
                        j = h * HALF + jj
                        x_chunk = x_tile[:, j, :]

                        # weighted[r, c] = sum_k expT[k,r] [CF|1][k,c]
                        ws_ps = ps_ws.tile([P, C + 2], f32, tag="ws")
                        nc.tensor.matmul(
                            ws_ps,
                            expT[:, bass.ts(jj, P)],
                            cf_aug,
                            start=True,
                            stop=True,
                        )

                        # normalize + residual (round-robin the engines:
                        # ACT and DVE split the PSUM-normalize, GPSIMD does
                        # most of the residual adds)
                        recip = work.tile([P, 1], f32, tag="recip")
                        nc.vector.reciprocal(recip, ws_ps[:, C : C + 1])
                        tmp = work.tile([P, C], f32, tag="tmp")
                        if j % 2 == 0:
                            nc.scalar.mul(tmp, ws_ps[:, :C], recip)
                        else:
                            nc.vector.tensor_scalar_mul(tmp, ws_ps[:, :C], recip)
                        nc.gpsimd.tensor_add(o_tile[:, j, :], tmp, x_chunk)

                nc.sync.dma_start(o_view[s, :, :HALF], o_tile[:, :HALF])
                nc.sync.dma_start(o_view[s, :, HALF:], o_tile[:, HALF:])

            ps_stack.close()

    return _split_multiwait_ctrl(nc, mybir)


def _get_nc():
    if "nc" not in _CACHE:
        _CACHE["nc"] = _build()
    return _CACHE["nc"]


def run(inputs, trace=False):
    from concourse.bass_utils import run_bass_kernel_spmd

    nc = _get_nc()
    pf = np.ascontiguousarray(np.asarray(inputs["point_features"], dtype=np.float32))
    cfeat = np.ascontiguousarray(
        np.asarray(inputs["centroid_features"], dtype=np.float32)
    )
    wp = np.ascontiguousarray(np.asarray(inputs["W_p"], dtype=np.float32))
    bp = np.ascontiguousarray(np.asarray(inputs["b_p"], dtype=np.float32))
    wc = np.ascontiguousarray(np.asarray(inputs["W_c"], dtype=np.float32))
    bc = np.ascontiguousarray(np.asarray(inputs["b_c"], dtype=np.float32))

    in_maps = [
        {"x": pf[b], "cf": cfeat[b], "wp": wp, "bp": bp, "wc": wc, "bc": bc}
        for b in range(B)
    ]
    res = run_bass_kernel_spmd(nc, in_maps, core_ids=list(range(B)), trace=trace)
    out = np.stack([res.results[b]["out"] for b in range(B)], axis=0)
    return out, res


def kernel(**inputs) -> np.ndarray:
    out, _ = run(inputs, trace=False)
    return out
